# revision 7
# baseline (speedup 1.0000x reference)
"""MoE (dense all-experts, top-2 gating) Trainium2 kernel.

Sharding: data-parallel over tokens. B*S = 8192 tokens -> 1024 tokens/core
on 8 NeuronCores; expert weights replicated per core.

Per-core program:
  - router logits in fp32 on the PE (x^T stripes as stationary, Wr moving),
    top-2 gating via DVE max8 + ACT sigmoid, dense gating scatter via
    is_equal masks; importance partial sums via ones-vector matmul.
  - FFN in fp32r (TF32-like, bf16-speed): for each (expert e, 512-wide
    F-chunk q): h = Gelu(x @ W1[e][:,q] + b1) with exact-gelu ACT eviction
    (bias per-partition), then out += g_e * (h @ W2[e][q,:]) with the gated
    accumulate fused into one DVE scalar_tensor_tensor per psum eviction.
  - b2 contribution: out_acc initialized from (gating^T)^T @ b2 via a PE
    transpose + small fp32 matmul.
aux_loss is finished on the host from per-core importance partials.
"""
import contextlib
import ctypes
import sys
import types

import numpy as np

for _p in ("/root/.axon_site", "/root/.axon_site/_ro/trn_rl_repo",
           "/root/.axon_site/_ro/pypackages", "/opt/trn_rl_repo"):
    if _p not in sys.path:
        sys.path.append(_p)

import concourse.bass as bass
import concourse.tile as tile
from concourse import mybir
from concourse.bass_utils import run_bass_kernel_spmd
from concourse.masks import make_identity
from concourse.vector_clock import ScopedClock, VectorClock
from contextlib import ExitStack

F32 = mybir.dt.float32
F32R = mybir.dt.float32r
AF = mybir.ActivationFunctionType
OP = mybir.AluOpType

B, S, H, F, E, TOPK = 4, 2048, 1024, 4096, 8, 2
NCORES = 8
TOK = B * S            # 8192
TPC = TOK // NCORES    # 1024 tokens per core
HC = H // 128          # 8 h-chunks
QF = 512               # F-chunk width
NQ = F // QF           # 8 chunks
FT = QF // 128         # 4 f-tiles per chunk
TT = TPC // 128        # 8 token tiles
TB = TPC // 512        # 2 token blocks of 512
HD = H // 512          # 2 output column halves

# ---------------------------------------------------------------------------
# walrus compat: this build caps sync waits at 1 per instruction (2 for
# EventSemaphore).  (a) kernel-tail drain must split its per-proc waits;
# (b) fp32/fp32r matmuls self-load weights (no separate Ldweights to spill
# waits to), so excess waits anywhere are moved onto InstNoOp carriers.
# ---------------------------------------------------------------------------
_noop_n = [0]


def _drain_split(self, tick_clock, wait_clock):
    vc = tick_clock.global_clock
    n = len(vc)
    for i in range(n):
        if vc[i] > 0:
            cv = VectorClock([0] * n)
            cv.require_at_least(i, vc[i])
            d = self.nc.sync.drain()
            wait_clock.add_sem_waits(d.ins, ScopedClock({None: cv}))
    self.nc.sync.drain()
    self.nc.all_engine_barrier()
    assert self.sems is not None
    popped = self.nc._tile_sem_poison_stack.pop()
    assert popped is self._sem_poison
    self.nc.clear_and_free_semaphores(list(self.sems.allocated().values()))
    self.nc.all_engine_barrier()


tile.TileContext._drain_and_barrier = _drain_split


def _split_excess_waits(nc):
    for fn in nc.m.functions:
        for bb in fn.blocks:
            out, changed = [], False
            for inst in list(bb.instructions):
                si = inst.sync_info
                waits = list(si.on_wait) if si is not None else []
                cap = 2 if type(inst).__name__ == "InstEventSemaphore" else 1
                if len(waits) > cap:
                    changed = True
                    for w in waits[:-cap]:
                        _noop_n[0] += 1
                        nop = mybir.InstNoOp(name=f"WN-{_noop_n[0]}")
                        nop.engine = inst.engine
                        nop.sync_info = mybir.SyncInfo(on_wait=[w], on_update=[])
                        out.append(nop)
                    inst.sync_info = mybir.SyncInfo(
                        on_wait=waits[-cap:], on_update=list(si.on_update))
                out.append(inst)
            if changed:
                bb.instructions = out


# ---------------------------------------------------------------------------
# NTFF profile hook (exec-time measurement under axon), used when trace=True.
# ---------------------------------------------------------------------------
def _install_ntff_hook():
    if "antenv.axon_hooks" in sys.modules:
        return
    try:
        import antenv
        lib = ctypes.CDLL("/opt/axon/libaxon_pjrt.so")
        if not hasattr(lib, "axon_start_nrt_profile"):
            return
        lib.axon_start_nrt_profile.argtypes = [ctypes.POINTER(ctypes.c_int64),
                                               ctypes.c_size_t]
        lib.axon_start_nrt_profile.restype = ctypes.c_int64
        lib.axon_stop_nrt_profile.argtypes = [ctypes.c_char_p]
        lib.axon_stop_nrt_profile.restype = ctypes.c_int64

        @contextlib.contextmanager
        def _hook(output_dir, device_ids):
            import jax
            jax.devices()
            if device_ids:
                ids = (ctypes.c_int64 * len(device_ids))(*device_ids)
                rc = lib.axon_start_nrt_profile(ids, len(device_ids))
            else:
                rc = lib.axon_start_nrt_profile(None, 0)
            if rc != 0:
                raise RuntimeError(f"axon_start_nrt_profile rc={rc}")
            try:
                yield
            finally:
                n = lib.axon_stop_nrt_profile(str(output_dir).encode())
                if n <= 0:
                    print(f"ntff profile rc={n}", file=sys.stderr)

        mod = types.ModuleType("antenv.axon_hooks")
        mod.get_axon_ntff_profile_hook = lambda: _hook
        mod.set_axon_ntff_profile_hook = lambda h: None
        sys.modules["antenv.axon_hooks"] = mod
        antenv.axon_hooks = mod
    except Exception as e:  # profiling is best-effort
        print(f"ntff hook unavailable: {e}", file=sys.stderr)


# ---------------------------------------------------------------------------
# Bass program (identical for all cores; per-core data differs via in_maps)
# ---------------------------------------------------------------------------
def _build():
    nc = bass.Bass()
    xT_d = nc.declare_dram_parameter("xT", [H, TPC], F32, isOutput=False)
    w1_d = nc.declare_dram_parameter("w1r", [E, H, F], F32R, isOutput=False)
    w2_d = nc.declare_dram_parameter("w2r", [E, F, H], F32R, isOutput=False)
    wr_d = nc.declare_dram_parameter("wr", [H, E], F32, isOutput=False)
    br_d = nc.declare_dram_parameter("br", [E], F32, isOutput=False)
    b1t_d = nc.declare_dram_parameter("b1t", [128, E * (F // 128)], F32,
                                      isOutput=False)
    b2_d = nc.declare_dram_parameter("b2", [E, H], F32, isOutput=False)
    out_d = nc.declare_dram_parameter("out", [TPC, H], F32, isOutput=True)
    imp_d = nc.declare_dram_parameter("imp", [1, E], F32, isOutput=True)

    with tile.TileContext(nc) as tc, ExitStack() as ctx:
        const = ctx.enter_context(tc.tile_pool(name="const", bufs=1))
        xpool = ctx.enter_context(tc.tile_pool(name="xpool", bufs=1))
        accp = ctx.enter_context(tc.tile_pool(name="accp", bufs=1))
        w1p = ctx.enter_context(tc.tile_pool(name="w1p", bufs=2))
        w2p = ctx.enter_context(tc.tile_pool(name="w2p", bufs=2))
        hp = ctx.enter_context(tc.tile_pool(name="hp", bufs=1))
        rp = ctx.enter_context(tc.tile_pool(name="rp", bufs=2))
        php = ctx.enter_context(tc.tile_pool(name="php", bufs=2, space="PSUM"))
        pop = ctx.enter_context(tc.tile_pool(name="pop", bufs=4, space="PSUM"))
        pim = ctx.enter_context(tc.tile_pool(name="pim", bufs=1, space="PSUM"))

        # ---- constants -----------------------------------------------------
        wr_sb = const.tile([128, HC, E], F32)
        for hc in range(HC):
            nc.sync.dma_start(out=wr_sb[:, hc, :],
                              in_=wr_d[hc * 128:(hc + 1) * 128, :])
        br_bc = const.tile([128, E], F32)
        nc.sync.dma_start(
            out=br_bc,
            in_=bass.AP(tensor=br_d.tensor if hasattr(br_d, "tensor") else br_d,
                        offset=0, ap=[[0, 128], [1, E]]))
        b1t_sb = const.tile([128, E * (F // 128)], F32)
        nc.sync.dma_start(out=b1t_sb, in_=b1t_d[:, :])
        b2_sb = const.tile([E, H], F32)
        nc.sync.dma_start(out=b2_sb, in_=b2_d[:, :])
        ones = const.tile([128, 1], F32)
        nc.vector.memset(ones, 1.0)
        wuf = const.tile([128, 512], F32)
        nc.vector.memset(wuf, 0.0)
        wu = const.tile([128, 512], F32R)
        nc.vector.tensor_copy(wu, wuf)
        for wi in range(44):
            ps_w = php.tile([128, 512], F32, name=f"psw{wi}", tag="ph")
            nc.tensor.matmul(ps_w[:, :], wu[:, 0:128], wu[:, :],
                             start=True, stop=True)
        ident = const.tile([128, 128], F32)
        make_identity(nc, ident)

        # ---- x^T stripes: fp32 (router) + fp32r (FFN) ----------------------
        xT = xpool.tile([128, HC, TPC], F32)
        for hc in range(HC):
            nc.sync.dma_start(out=xT[:, hc, :],
                              in_=xT_d[hc * 128:(hc + 1) * 128, :])
        xTr = xpool.tile([128, HC, TPC], F32R)
        for hc in range(HC):
            nc.vector.tensor_copy(xTr[:, hc, :], xT[:, hc, :])

        out_acc = accp.tile([128, TT, H], F32)
        gat = accp.tile([128, TT, E], F32)

        # ---- router + gating + out_acc init (b2 term) ----------------------
        imp_ps = pim.tile([1, E], F32)
        for tt in range(TT):
            ps_r = php.tile([128, E], F32, name=f"psr{tt}", tag="ph")
            for hc in range(HC):
                nc.tensor.matmul(ps_r[:, :],
                                 xT[:, hc, tt * 128:(tt + 1) * 128],
                                 wr_sb[:, hc, :],
                                 start=(hc == 0), stop=(hc == HC - 1))
            lg = rp.tile([128, E], F32, name=f"lg{tt}", tag="lg")
            nc.vector.tensor_tensor(lg, ps_r[:, :], br_bc, op=OP.add)

            srt = rp.tile([128, 8], F32, name=f"srt{tt}", tag="srt")
            nc.vector.max(out=srt, in_=lg)
            m1 = srt[:, 0:1]
            m2 = srt[:, 1:2]

            sc = rp.tile([128, 8], F32, name=f"sc{tt}", tag="sc")
            negm1 = sc[:, 0:1]
            diff = sc[:, 1:2]
            w1s = sc[:, 2:3]
            w2s = sc[:, 3:4]
            den = sc[:, 4:5]
            rden = sc[:, 5:6]
            nc.vector.tensor_scalar_mul(negm1, m1, -1.0)
            nc.vector.tensor_sub(diff, m1, m2)
            nc.scalar.activation(w1s, diff, AF.Sigmoid, bias=0.0, scale=1.0)
            nc.scalar.activation(w2s, diff, AF.Sigmoid, bias=0.0, scale=-1.0)

            pex = rp.tile([128, E], F32, name=f"pex{tt}", tag="pex")
            nc.scalar.activation(pex, lg, AF.Exp, bias=negm1, scale=1.0)
            nc.vector.reduce_sum(den, pex, axis=mybir.AxisListType.X)
            nc.vector.reciprocal(rden, den)
            probs = rp.tile([128, E], F32, name=f"pr{tt}", tag="pr")
            nc.vector.tensor_scalar_mul(probs, pex, rden)
            nc.tensor.matmul(imp_ps[:, :], ones[:, :], probs[:, :],
                             start=(tt == 0), stop=(tt == TT - 1))

            eq1 = rp.tile([128, E], F32, name=f"eq1{tt}", tag="eq1")
            eq2 = rp.tile([128, E], F32, name=f"eq2{tt}", tag="eq2")
            nc.vector.tensor_scalar(eq1, lg, m1, None, op0=OP.is_equal)
            nc.vector.tensor_scalar(eq2, lg, m2, None, op0=OP.is_equal)
            g = gat[:, tt, :]
            nc.vector.tensor_scalar_mul(g, eq1, w1s)
            nc.vector.scalar_tensor_tensor(out=g, in0=eq2, scalar=w2s, in1=g,
                                           op0=OP.mult, op1=OP.add)

        # b2 term in a second pass: out_acc[tt] = (gating^T)^T @ b2 --
        # keeps the PE from stalling on each tt's DVE/ACT gating chain
        for tt in range(TT):
            ps_t = pop.tile([8, 128], F32, name=f"pst{tt}", tag="po")
            nc.tensor.transpose(ps_t[:, :], gat[:, tt, :], ident[:, :])
            gT = rp.tile([8, 128], F32, name=f"gT{tt}", tag="gT")
            nc.vector.tensor_copy(gT, ps_t[:, :])
            for hd in range(HD):
                ps_b = pop.tile([128, 512], F32, name=f"psb{tt}_{hd}", tag="po")
                nc.tensor.matmul(ps_b[:, :], gT[:, :],
                                 b2_sb[:, hd * 512:(hd + 1) * 512],
                                 start=True, stop=True)
                nc.scalar.activation(out_acc[:, tt, hd * 512:(hd + 1) * 512],
                                     ps_b[:, :], AF.Copy, bias=0.0, scale=1.0)

        imp_sb = const.tile([1, E], F32)
        nc.vector.tensor_copy(imp_sb, imp_ps[:, :])
        nc.sync.dma_start(out=imp_d[:, :], in_=imp_sb)

        # ---- FFN: experts x F-chunks --------------------------------------
        for e in range(E):
            for q in range(NQ):
                w1t = w1p.tile([128, HC, QF], F32R, name=f"w1_{e}_{q}",
                               tag="w1")
                for hc in range(HC):
                    nc.sync.dma_start(
                        out=w1t[:, hc, :],
                        in_=w1_d[e, hc * 128:(hc + 1) * 128,
                                 q * QF:(q + 1) * QF])
                w2t = w2p.tile([128, FT, H], F32R, name=f"w2_{e}_{q}",
                               tag="w2")
                for fc in range(FT):
                    nc.sync.dma_start(
                        out=w2t[:, fc, :],
                        in_=w2_d[e, q * QF + fc * 128:q * QF + (fc + 1) * 128,
                                 :])
                ht = hp.tile([128, FT, TPC], F32R, name=f"h_{e}_{q}", tag="h")

                for ft in range(FT):
                    for tb in range(TB):
                        ps = php.tile([128, 512], F32,
                                      name=f"ph{e}_{q}_{ft}_{tb}", tag="ph")
                        for hc in range(HC):
                            nc.tensor.matmul(
                                ps[:, :],
                                w1t[:, hc, ft * 128:(ft + 1) * 128],
                                xTr[:, hc, tb * 512:(tb + 1) * 512],
                                start=(hc == 0), stop=(hc == HC - 1))
                        bcol = b1t_sb[:, e * (F // 128) + q * FT + ft:
                                      e * (F // 128) + q * FT + ft + 1]
                        nc.scalar.activation(ht[:, ft, tb * 512:(tb + 1) * 512],
                                             ps[:, :], AF.Gelu,
                                             bias=bcol, scale=1.0)

                for tt in range(TT):
                    for hd in range(HD):
                        po = pop.tile([128, 512], F32,
                                      name=f"po{e}_{q}_{tt}_{hd}", tag="po")
                        for fc in range(FT):
                            nc.tensor.matmul(
                                po[:, :],
                                ht[:, fc, tt * 128:(tt + 1) * 128],
                                w2t[:, fc, hd * 512:(hd + 1) * 512],
                                start=(fc == 0), stop=(fc == FT - 1))
                        oslice = out_acc[:, tt, hd * 512:(hd + 1) * 512]
                        nc.vector.scalar_tensor_tensor(
                            out=oslice, in0=po[:, :],
                            scalar=gat[:, tt, e:e + 1], in1=oslice,
                            op0=OP.mult, op1=OP.add)

        # ---- write out -----------------------------------------------------
        for tt in range(TT):
            nc.sync.dma_start(out=out_d[tt * 128:(tt + 1) * 128, :],
                              in_=out_acc[:, tt, :])

    _split_excess_waits(nc)
    return nc


_CACHE = {}


def _get_nc():
    if "nc" not in _CACHE:
        _CACHE["nc"] = _build()
    return _CACHE["nc"]


def kernel(x, W1, b1, W2, b2, Wr, br, _trace=False):
    x = np.asarray(x, dtype=np.float32)
    W1 = np.ascontiguousarray(np.asarray(W1, dtype=np.float32))
    W2 = np.ascontiguousarray(np.asarray(W2, dtype=np.float32))
    b1 = np.asarray(b1, dtype=np.float32)
    b2 = np.ascontiguousarray(np.asarray(b2, dtype=np.float32))
    Wr = np.ascontiguousarray(np.asarray(Wr, dtype=np.float32))
    br = np.asarray(br, dtype=np.float32)

    b1t = np.ascontiguousarray(
        b1.reshape(E, F // 128, 128).transpose(2, 0, 1).reshape(128, -1))
    x_flat = x.reshape(TOK, H)

    if _trace:
        _install_ntff_hook()

    nc = _get_nc()
    in_maps = []
    for c in range(NCORES):
        xT_c = np.ascontiguousarray(x_flat[c * TPC:(c + 1) * TPC].T)
        in_maps.append({
            "xT": xT_c, "w1r": W1, "w2r": W2, "wr": Wr, "br": br,
            "b1t": b1t, "b2": b2,
        })
    res = run_bass_kernel_spmd(nc, in_maps, list(range(NCORES)),
                               trace=_trace)
    out = np.concatenate([res.results[c]["out"] for c in range(NCORES)],
                         axis=0).reshape(B, S, H)
    imp = np.sum([res.results[c]["imp"][0] for c in range(NCORES)], axis=0)
    aux = np.float32(E * np.sum((imp / np.float32(TOK)) ** 2))
    if _trace:
        kernel.last_exec_time_ns = res.exec_time_ns
    return out, aux


# revision 8
# speedup vs baseline: 1.0000x; 1.0000x over previous
"""MoE (dense all-experts, top-2 gating) Trainium2 kernel.

Sharding: data-parallel over tokens. B*S = 8192 tokens -> 1024 tokens/core
on 8 NeuronCores; expert weights replicated per core.

Per-core program:
  - router logits in fp32 on the PE (x^T stripes as stationary, Wr moving),
    top-2 gating via DVE max8 + ACT sigmoid, dense gating scatter via
    is_equal masks; importance partial sums via ones-vector matmul.
  - FFN in fp32r (TF32-like, bf16-speed): for each (expert e, 512-wide
    F-chunk q): h = Gelu(x @ W1[e][:,q] + b1) with exact-gelu ACT eviction
    (bias per-partition), then out += g_e * (h @ W2[e][q,:]) with the gated
    accumulate fused into one DVE scalar_tensor_tensor per psum eviction.
  - b2 contribution: out_acc initialized from (gating^T)^T @ b2 via a PE
    transpose + small fp32 matmul.
aux_loss is finished on the host from per-core importance partials.
"""
import contextlib
import ctypes
import sys
import types

import numpy as np

for _p in ("/root/.axon_site", "/root/.axon_site/_ro/trn_rl_repo",
           "/root/.axon_site/_ro/pypackages", "/opt/trn_rl_repo"):
    if _p not in sys.path:
        sys.path.append(_p)

import concourse.bass as bass
import concourse.tile as tile
from concourse import mybir
from concourse.bass_utils import run_bass_kernel_spmd
from concourse.masks import make_identity
from concourse.vector_clock import ScopedClock, VectorClock
from contextlib import ExitStack

F32 = mybir.dt.float32
F32R = mybir.dt.float32r
AF = mybir.ActivationFunctionType
OP = mybir.AluOpType

B, S, H, F, E, TOPK = 4, 2048, 1024, 4096, 8, 2
NCORES = 8
TOK = B * S            # 8192
TPC = TOK // NCORES    # 1024 tokens per core
HC = H // 128          # 8 h-chunks
QF = 512               # F-chunk width
NQ = F // QF           # 8 chunks
FT = QF // 128         # 4 f-tiles per chunk
TT = TPC // 128        # 8 token tiles
TB = TPC // 512        # 2 token blocks of 512
HD = H // 512          # 2 output column halves

# ---------------------------------------------------------------------------
# walrus compat: this build caps sync waits at 1 per instruction (2 for
# EventSemaphore).  (a) kernel-tail drain must split its per-proc waits;
# (b) fp32/fp32r matmuls self-load weights (no separate Ldweights to spill
# waits to), so excess waits anywhere are moved onto InstNoOp carriers.
# ---------------------------------------------------------------------------
_noop_n = [0]


def _drain_split(self, tick_clock, wait_clock):
    vc = tick_clock.global_clock
    n = len(vc)
    for i in range(n):
        if vc[i] > 0:
            cv = VectorClock([0] * n)
            cv.require_at_least(i, vc[i])
            d = self.nc.sync.drain()
            wait_clock.add_sem_waits(d.ins, ScopedClock({None: cv}))
    self.nc.sync.drain()
    self.nc.all_engine_barrier()
    assert self.sems is not None
    popped = self.nc._tile_sem_poison_stack.pop()
    assert popped is self._sem_poison
    self.nc.clear_and_free_semaphores(list(self.sems.allocated().values()))
    self.nc.all_engine_barrier()


tile.TileContext._drain_and_barrier = _drain_split


def _split_excess_waits(nc):
    for fn in nc.m.functions:
        for bb in fn.blocks:
            out, changed = [], False
            for inst in list(bb.instructions):
                si = inst.sync_info
                waits = list(si.on_wait) if si is not None else []
                cap = 2 if type(inst).__name__ == "InstEventSemaphore" else 1
                if len(waits) > cap:
                    changed = True
                    for w in waits[:-cap]:
                        _noop_n[0] += 1
                        nop = mybir.InstNoOp(name=f"WN-{_noop_n[0]}")
                        nop.engine = inst.engine
                        nop.sync_info = mybir.SyncInfo(on_wait=[w], on_update=[])
                        out.append(nop)
                    inst.sync_info = mybir.SyncInfo(
                        on_wait=waits[-cap:], on_update=list(si.on_update))
                out.append(inst)
            if changed:
                bb.instructions = out


# ---------------------------------------------------------------------------
# NTFF profile hook (exec-time measurement under axon), used when trace=True.
# ---------------------------------------------------------------------------
def _install_ntff_hook():
    if "antenv.axon_hooks" in sys.modules:
        return
    try:
        import antenv
        lib = ctypes.CDLL("/opt/axon/libaxon_pjrt.so")
        if not hasattr(lib, "axon_start_nrt_profile"):
            return
        lib.axon_start_nrt_profile.argtypes = [ctypes.POINTER(ctypes.c_int64),
                                               ctypes.c_size_t]
        lib.axon_start_nrt_profile.restype = ctypes.c_int64
        lib.axon_stop_nrt_profile.argtypes = [ctypes.c_char_p]
        lib.axon_stop_nrt_profile.restype = ctypes.c_int64

        @contextlib.contextmanager
        def _hook(output_dir, device_ids):
            import jax
            jax.devices()
            if device_ids:
                ids = (ctypes.c_int64 * len(device_ids))(*device_ids)
                rc = lib.axon_start_nrt_profile(ids, len(device_ids))
            else:
                rc = lib.axon_start_nrt_profile(None, 0)
            if rc != 0:
                raise RuntimeError(f"axon_start_nrt_profile rc={rc}")
            try:
                yield
            finally:
                n = lib.axon_stop_nrt_profile(str(output_dir).encode())
                if n <= 0:
                    print(f"ntff profile rc={n}", file=sys.stderr)

        mod = types.ModuleType("antenv.axon_hooks")
        mod.get_axon_ntff_profile_hook = lambda: _hook
        mod.set_axon_ntff_profile_hook = lambda h: None
        sys.modules["antenv.axon_hooks"] = mod
        antenv.axon_hooks = mod
    except Exception as e:  # profiling is best-effort
        print(f"ntff hook unavailable: {e}", file=sys.stderr)


# ---------------------------------------------------------------------------
# Bass program (identical for all cores; per-core data differs via in_maps)
# ---------------------------------------------------------------------------
def _build():
    nc = bass.Bass()
    xT_d = nc.declare_dram_parameter("xT", [H, TPC], F32, isOutput=False)
    w1_d = nc.declare_dram_parameter("w1r", [E, H, F], F32R, isOutput=False)
    w2_d = nc.declare_dram_parameter("w2r", [E, F, H], F32R, isOutput=False)
    wr_d = nc.declare_dram_parameter("wr", [H, E], F32, isOutput=False)
    br_d = nc.declare_dram_parameter("br", [E], F32, isOutput=False)
    b1t_d = nc.declare_dram_parameter("b1t", [128, E * (F // 128)], F32,
                                      isOutput=False)
    b2_d = nc.declare_dram_parameter("b2", [E, H], F32, isOutput=False)
    out_d = nc.declare_dram_parameter("out", [TPC, H], F32, isOutput=True)
    imp_d = nc.declare_dram_parameter("imp", [1, E], F32, isOutput=True)

    with tile.TileContext(nc) as tc, ExitStack() as ctx:
        const = ctx.enter_context(tc.tile_pool(name="const", bufs=1))
        xpool = ctx.enter_context(tc.tile_pool(name="xpool", bufs=1))
        accp = ctx.enter_context(tc.tile_pool(name="accp", bufs=1))
        w1p = ctx.enter_context(tc.tile_pool(name="w1p", bufs=2))
        w2p = ctx.enter_context(tc.tile_pool(name="w2p", bufs=2))
        hp = ctx.enter_context(tc.tile_pool(name="hp", bufs=1))
        rp = ctx.enter_context(tc.tile_pool(name="rp", bufs=2))
        php = ctx.enter_context(tc.tile_pool(name="php", bufs=3, space="PSUM"))
        pop = ctx.enter_context(tc.tile_pool(name="pop", bufs=4, space="PSUM"))
        pim = ctx.enter_context(tc.tile_pool(name="pim", bufs=1, space="PSUM"))

        # ---- constants -----------------------------------------------------
        wr_sb = const.tile([128, HC, E], F32)
        for hc in range(HC):
            nc.sync.dma_start(out=wr_sb[:, hc, :],
                              in_=wr_d[hc * 128:(hc + 1) * 128, :])
        br_bc = const.tile([128, E], F32)
        nc.sync.dma_start(
            out=br_bc,
            in_=bass.AP(tensor=br_d.tensor if hasattr(br_d, "tensor") else br_d,
                        offset=0, ap=[[0, 128], [1, E]]))
        b1t_sb = const.tile([128, E * (F // 128)], F32)
        nc.sync.dma_start(out=b1t_sb, in_=b1t_d[:, :])
        b2_sb = const.tile([E, H], F32)
        nc.sync.dma_start(out=b2_sb, in_=b2_d[:, :])
        ones = const.tile([128, 1], F32)
        nc.vector.memset(ones, 1.0)
        wuf = const.tile([128, 512], F32)
        nc.vector.memset(wuf, 0.0)
        wu = const.tile([128, 512], F32R)
        nc.vector.tensor_copy(wu, wuf)
        for wi in range(44):
            ps_w = php.tile([128, 512], F32, name=f"psw{wi}", tag="ph")
            nc.tensor.matmul(ps_w[:, :], wu[:, 0:128], wu[:, :],
                             start=True, stop=True)
        ident = const.tile([128, 128], F32)
        make_identity(nc, ident)

        # ---- x^T stripes: fp32 (router) + fp32r (FFN) ----------------------
        xT = xpool.tile([128, HC, TPC], F32)
        for hc in range(HC):
            nc.sync.dma_start(out=xT[:, hc, :],
                              in_=xT_d[hc * 128:(hc + 1) * 128, :])
        xTr = xpool.tile([128, HC, TPC], F32R)
        for hc in range(HC):
            nc.vector.tensor_copy(xTr[:, hc, :], xT[:, hc, :])

        out_acc = accp.tile([128, TT, H], F32)
        gat = accp.tile([128, TT, E], F32)

        # ---- router + gating + out_acc init (b2 term) ----------------------
        imp_ps = pim.tile([1, E], F32)
        for tt in range(TT):
            ps_r = php.tile([128, E], F32, name=f"psr{tt}", tag="ph")
            for hc in range(HC):
                nc.tensor.matmul(ps_r[:, :],
                                 xT[:, hc, tt * 128:(tt + 1) * 128],
                                 wr_sb[:, hc, :],
                                 start=(hc == 0), stop=(hc == HC - 1))
            lg = rp.tile([128, E], F32, name=f"lg{tt}", tag="lg")
            nc.vector.tensor_tensor(lg, ps_r[:, :], br_bc, op=OP.add)

            srt = rp.tile([128, 8], F32, name=f"srt{tt}", tag="srt")
            nc.vector.max(out=srt, in_=lg)
            m1 = srt[:, 0:1]
            m2 = srt[:, 1:2]

            sc = rp.tile([128, 8], F32, name=f"sc{tt}", tag="sc")
            negm1 = sc[:, 0:1]
            diff = sc[:, 1:2]
            w1s = sc[:, 2:3]
            w2s = sc[:, 3:4]
            den = sc[:, 4:5]
            rden = sc[:, 5:6]
            nc.vector.tensor_scalar_mul(negm1, m1, -1.0)
            nc.vector.tensor_sub(diff, m1, m2)
            nc.scalar.activation(w1s, diff, AF.Sigmoid, bias=0.0, scale=1.0)
            nc.scalar.activation(w2s, diff, AF.Sigmoid, bias=0.0, scale=-1.0)

            pex = rp.tile([128, E], F32, name=f"pex{tt}", tag="pex")
            nc.scalar.activation(pex, lg, AF.Exp, bias=negm1, scale=1.0)
            nc.vector.reduce_sum(den, pex, axis=mybir.AxisListType.X)
            nc.vector.reciprocal(rden, den)
            probs = rp.tile([128, E], F32, name=f"pr{tt}", tag="pr")
            nc.vector.tensor_scalar_mul(probs, pex, rden)
            nc.tensor.matmul(imp_ps[:, :], ones[:, :], probs[:, :],
                             start=(tt == 0), stop=(tt == TT - 1))

            eq1 = rp.tile([128, E], F32, name=f"eq1{tt}", tag="eq1")
            eq2 = rp.tile([128, E], F32, name=f"eq2{tt}", tag="eq2")
            nc.vector.tensor_scalar(eq1, lg, m1, None, op0=OP.is_equal)
            nc.vector.tensor_scalar(eq2, lg, m2, None, op0=OP.is_equal)
            g = gat[:, tt, :]
            nc.vector.tensor_scalar_mul(g, eq1, w1s)
            nc.vector.scalar_tensor_tensor(out=g, in0=eq2, scalar=w2s, in1=g,
                                           op0=OP.mult, op1=OP.add)

        # b2 term in a second pass: out_acc[tt] = (gating^T)^T @ b2 --
        # keeps the PE from stalling on each tt's DVE/ACT gating chain
        for tt in range(TT):
            ps_t = pop.tile([8, 128], F32, name=f"pst{tt}", tag="po")
            nc.tensor.transpose(ps_t[:, :], gat[:, tt, :], ident[:, :])
            gT = rp.tile([8, 128], F32, name=f"gT{tt}", tag="gT")
            nc.vector.tensor_copy(gT, ps_t[:, :])
            for hd in range(HD):
                ps_b = pop.tile([128, 512], F32, name=f"psb{tt}_{hd}", tag="po")
                nc.tensor.matmul(ps_b[:, :], gT[:, :],
                                 b2_sb[:, hd * 512:(hd + 1) * 512],
                                 start=True, stop=True)
                nc.scalar.activation(out_acc[:, tt, hd * 512:(hd + 1) * 512],
                                     ps_b[:, :], AF.Copy, bias=0.0, scale=1.0)

        imp_sb = const.tile([1, E], F32)
        nc.vector.tensor_copy(imp_sb, imp_ps[:, :])
        nc.sync.dma_start(out=imp_d[:, :], in_=imp_sb)

        # ---- FFN: experts x F-chunks --------------------------------------
        for e in range(E):
            for q in range(NQ):
                w1t = w1p.tile([128, HC, QF], F32R, name=f"w1_{e}_{q}",
                               tag="w1")
                for hc in range(HC):
                    nc.sync.dma_start(
                        out=w1t[:, hc, :],
                        in_=w1_d[e, hc * 128:(hc + 1) * 128,
                                 q * QF:(q + 1) * QF])
                w2t = w2p.tile([128, FT, H], F32R, name=f"w2_{e}_{q}",
                               tag="w2")
                for fc in range(FT):
                    nc.sync.dma_start(
                        out=w2t[:, fc, :],
                        in_=w2_d[e, q * QF + fc * 128:q * QF + (fc + 1) * 128,
                                 :])
                ht = hp.tile([128, FT, TPC], F32R, name=f"h_{e}_{q}", tag="h")

                for ft in range(FT):
                    for tb in range(TB):
                        ps = php.tile([128, 512], F32,
                                      name=f"ph{e}_{q}_{ft}_{tb}", tag="ph")
                        for hc in range(HC):
                            nc.tensor.matmul(
                                ps[:, :],
                                w1t[:, hc, ft * 128:(ft + 1) * 128],
                                xTr[:, hc, tb * 512:(tb + 1) * 512],
                                start=(hc == 0), stop=(hc == HC - 1))
                        bcol = b1t_sb[:, e * (F // 128) + q * FT + ft:
                                      e * (F // 128) + q * FT + ft + 1]
                        nc.scalar.activation(ht[:, ft, tb * 512:(tb + 1) * 512],
                                             ps[:, :], AF.Gelu,
                                             bias=bcol, scale=1.0)

                for tt in range(TT):
                    for hd in range(HD):
                        po = pop.tile([128, 512], F32,
                                      name=f"po{e}_{q}_{tt}_{hd}", tag="po")
                        for fc in range(FT):
                            nc.tensor.matmul(
                                po[:, :],
                                ht[:, fc, tt * 128:(tt + 1) * 128],
                                w2t[:, fc, hd * 512:(hd + 1) * 512],
                                start=(fc == 0), stop=(fc == FT - 1))
                        oslice = out_acc[:, tt, hd * 512:(hd + 1) * 512]
                        nc.vector.scalar_tensor_tensor(
                            out=oslice, in0=po[:, :],
                            scalar=gat[:, tt, e:e + 1], in1=oslice,
                            op0=OP.mult, op1=OP.add)

        # ---- write out -----------------------------------------------------
        for tt in range(TT):
            nc.sync.dma_start(out=out_d[tt * 128:(tt + 1) * 128, :],
                              in_=out_acc[:, tt, :])

    _split_excess_waits(nc)
    return nc


_CACHE = {}


def _get_nc():
    if "nc" not in _CACHE:
        _CACHE["nc"] = _build()
    return _CACHE["nc"]


def kernel(x, W1, b1, W2, b2, Wr, br, _trace=False):
    x = np.asarray(x, dtype=np.float32)
    W1 = np.ascontiguousarray(np.asarray(W1, dtype=np.float32))
    W2 = np.ascontiguousarray(np.asarray(W2, dtype=np.float32))
    b1 = np.asarray(b1, dtype=np.float32)
    b2 = np.ascontiguousarray(np.asarray(b2, dtype=np.float32))
    Wr = np.ascontiguousarray(np.asarray(Wr, dtype=np.float32))
    br = np.asarray(br, dtype=np.float32)

    b1t = np.ascontiguousarray(
        b1.reshape(E, F // 128, 128).transpose(2, 0, 1).reshape(128, -1))
    x_flat = x.reshape(TOK, H)

    if _trace:
        _install_ntff_hook()

    nc = _get_nc()
    in_maps = []
    for c in range(NCORES):
        xT_c = np.ascontiguousarray(x_flat[c * TPC:(c + 1) * TPC].T)
        in_maps.append({
            "xT": xT_c, "w1r": W1, "w2r": W2, "wr": Wr, "br": br,
            "b1t": b1t, "b2": b2,
        })
    res = run_bass_kernel_spmd(nc, in_maps, list(range(NCORES)),
                               trace=_trace)
    out = np.concatenate([res.results[c]["out"] for c in range(NCORES)],
                         axis=0).reshape(B, S, H)
    imp = np.sum([res.results[c]["imp"][0] for c in range(NCORES)], axis=0)
    aux = np.float32(E * np.sum((imp / np.float32(TOK)) ** 2))
    if _trace:
        kernel.last_exec_time_ns = res.exec_time_ns
    return out, aux
